# revision 25
# baseline (speedup 1.0000x reference)
"""Trainium2 Bass kernel: batched Ising energies E_b = s_b^T J s_b.

state: [1024, 2048] float32 in {0,1};  J: [2048, 2048] float32.
Returns energies [1024] float32.

Because s_i^2 = 1, E = s^T J s = s^T A s where A folds the symmetric
part of J into the upper block-triangle (A_ij = J_ij + J_ji for i<j,
A_ii = J_ii, zero below).  That halves both the matmul work and the
J bytes moved; A ships as bf16 (rel err ~2.5e-3, tolerance 2e-2).

Sharding (8 cores): 2 batch halves x 4 column groups.  Core (r, c)
owns column tiles {4j+r : j=0..3} of A for batch half c.  For a single
SPMD program across cores with different triangle supports, slot j
accumulates its ctile over a fixed cap of 4(j+1) contraction tiles
(ragged edge zero-padded on the host) and ktile storage order is
permuted per core so ctile 4j+r's spin rows always sit at position
4j+3 (uniform epilogue addressing).

Per core: A tiles are the stationary operand; spins ship directly as
fp8e4 {-1,+1} (exact, 1 byte) and stream as the moving operand -- the
PE accepts bf16 stationary x fp8 moving, so no on-chip expand at all.
psum[j] = g = A_block^T s.  Epilogue per slot: m = psum[j] * spin rows
(DVE, fp8 operand), then a ones-vector matmul reduces m across
partitions into a [1, 512] psum accumulator; one output DMA.

Both input streams are HOST-INTERLEAVED into ONE DRAM tensor in exact
consumption order (per contraction step: 512 B of spins then the
step's A tiles), so each chunk is a single DMA -- halving the number
of completion-semaphore increments, whose ~120 ns global drain rate
is what actually paces chunk consumability.  Chunks alternate between
the two HWDGE rings; on-chip consumers read dtype-bitcast slices of
the one combined SBUF buffer.  Dummy matmuls warm the PE clock gate
while the first chunk lands.
"""

import sys

if "/opt/trn_rl_repo" not in sys.path:
    sys.path.insert(0, "/opt/trn_rl_repo")

import numpy as np
import ml_dtypes

B, N = 1024, 2048
P = 128
KT = N // P          # 16 contraction/column tiles
R, C = 4, 2          # column groups x batch halves
BH = B // C          # 512 samples per core
S_CAP = [4, 8, 12, 16]   # per-slot ktile caps (uniform across cores)
N_WARM = 70          # PE clock-gate warmup matmuls
WARM_N = 64
CHUNKS = [(0, 2), (2, 6), (6, 16)]   # step ranges per input DMA

SU_BYTES = BH                        # fp8 spins per step per partition
AT_BYTES = 2 * P                     # one bf16 A tile per partition


def _n_act(s):
    return sum(1 for cap in S_CAP if cap > s)


OFF = [0]
for _s in range(KT):
    OFF.append(OFF[-1] + SU_BYTES + AT_BYTES * _n_act(_s))
TOTAL = OFF[KT]

_cache = {}


def _build_program():
    import concourse.bacc as bacc
    import concourse.mybir as mybir
    import concourse.tile as tile

    bf16 = mybir.dt.bfloat16
    f32 = mybir.dt.float32
    fp8 = mybir.dt.float8e4
    u8 = mybir.dt.uint8

    nc = bacc.Bacc("TRN2", target_bir_lowering=False, debug=False, num_devices=R * C)

    comb_ext = nc.dram_tensor("comb", [P, TOTAL], u8, kind="ExternalInput").ap()
    out_ext = nc.dram_tensor("part", [1, BH], f32, kind="ExternalOutput").ap()

    with tile.TileContext(nc) as tc:
        with (
            tc.tile_pool(name="persist", bufs=1) as persist,
            tc.tile_pool(name="work", bufs=1) as work,
            tc.tile_pool(name="psum", bufs=1, space="PSUM") as psum_pool,
            tc.tile_pool(name="warmps", bufs=1, space="PSUM") as warm_pool,
        ):
            comb_t = persist.tile([P, TOTAL], u8)
            ones_t = persist.tile([P, 1], bf16)
            warm_src = persist.tile([P, P], bf16)
            red_sb = persist.tile([1, BH], f32)
            m_t = [work.tile([P, BH], bf16, name=f"m_{j}") for j in range(4)]

            ps = [psum_pool.tile([P, BH], f32, name=f"ps_{j}") for j in range(4)]
            ep = psum_pool.tile([1, BH], f32, name="ep")
            warm_ps = warm_pool.tile([P, WARM_N], f32)

            def su_view(s):
                return comb_t[:, OFF[s]:OFF[s] + SU_BYTES].bitcast(fp8)

            def a_view(s, rank):
                a0 = OFF[s] + SU_BYTES + AT_BYTES * rank
                return comb_t[:, a0:a0 + AT_BYTES].bitcast(bf16)

            # constants via gpsimd (released early in the preamble)
            nc.gpsimd.memset(warm_src[:], 0.0)
            nc.gpsimd.memset(ones_t[:], 1.0)

            # PE warmup: dummy matmuls keep the HAM activity window busy
            # while the first input chunk lands.
            for _ in range(N_WARM):
                nc.tensor.matmul(
                    warm_ps, lhsT=warm_src[:, :P], rhs=warm_src[:, :WARM_N],
                    start=True, stop=True,
                )

            # one DMA per chunk, alternating HWDGE rings, consumption
            # order.  The first (latency-critical) chunk is split by
            # partition halves across BOTH rings: each half uses 8 SDMA
            # engines, so the slowest-engine completion skew shrinks and
            # both rings push its bytes concurrently.
            (k0, k1) = CHUNKS[0]
            nc.sync.dma_start(
                out=comb_t[0:64, OFF[k0]:OFF[k1]],
                in_=comb_ext[0:64, OFF[k0]:OFF[k1]],
                single_packet=True,
            )
            nc.scalar.dma_start(
                out=comb_t[64:128, OFF[k0]:OFF[k1]],
                in_=comb_ext[64:128, OFF[k0]:OFF[k1]],
                single_packet=True,
            )
            for ci, (k0, k1) in enumerate(CHUNKS[1:]):
                eng = nc.scalar if ci % 2 == 0 else nc.sync
                eng.dma_start(
                    out=comb_t[:, OFF[k0]:OFF[k1]],
                    in_=comb_ext[:, OFF[k0]:OFF[k1]],
                    single_packet=True,
                )

            def epilogue(j):
                # m = ps[j] * spin rows of ctile j (position 4j+3), then
                # reduce across partitions via a ones-vector matmul.
                nc.vector.scalar_tensor_tensor(
                    m_t[j][:],
                    ps[j][:],
                    1.0,
                    su_view(4 * j + 3),
                    mybir.AluOpType.mult,
                    mybir.AluOpType.mult,
                )
                nc.tensor.matmul(
                    ep,
                    lhsT=ones_t[:],
                    rhs=m_t[j][:],
                    start=(j == 0),
                    stop=(j == 3),
                )
                if j == 3:
                    # PSUM -> SBUF (DMA cannot read PSUM); halves run on
                    # vector and scalar concurrently.
                    nc.vector.tensor_scalar(
                        red_sb[:, :BH // 2], ep[:, :BH // 2], 1.0, 0.0,
                        mybir.AluOpType.mult, mybir.AluOpType.add,
                    )
                    nc.scalar.activation(
                        red_sb[:, BH // 2:], ep[:, BH // 2:],
                        mybir.ActivationFunctionType.Copy,
                    )

            for s in range(KT):
                rank = 0
                for j in range(4):
                    if S_CAP[j] <= s:
                        continue
                    nc.tensor.matmul(
                        ps[j],
                        lhsT=a_view(s, rank),
                        rhs=su_view(s),
                        start=(s == 0),
                        stop=(s == S_CAP[j] - 1),
                    )
                    rank += 1
                # emit each finished slot's epilogue one step late so the
                # ones-matmul's wait on the DVE multiply is already
                # satisfied when the PE reaches it (no exposed gap).
                for j in range(4):
                    if s == min(S_CAP[j], KT - 1):
                        epilogue(j)

            nc.sync.dma_start(out=out_ext, in_=red_sb[:])

    nc.compile()
    return nc


def _make_in_maps(state, J):
    bf16 = ml_dtypes.bfloat16
    fp8 = ml_dtypes.float8_e4m3
    state = np.asarray(state, dtype=np.float32)
    J = np.asarray(J, dtype=np.float32)

    # fold the symmetric part into the upper block-triangle
    A = np.triu(J + J.T, 1) + np.diag(np.diag(J))
    A = A.astype(bf16)
    sp8 = np.where(state > 0.5, np.float32(1.0), np.float32(-1.0)).astype(fp8)

    in_maps = []
    placement = []
    for core in range(R * C):
        r, c = divmod(core, C)
        kt_of_pos = []
        for g in range(4):
            grp = [x for x in range(4 * g, 4 * g + 4) if x != 4 * g + r]
            kt_of_pos += grp + [4 * g + r]
        ctile = [4 * j + r for j in range(4)]

        sm = sp8[c * BH:(c + 1) * BH]               # [BH, N]
        comb = np.zeros((P, TOTAL), dtype=np.uint8)
        for s in range(KT):
            kt = kt_of_pos[s]
            su = np.ascontiguousarray(sm[:, kt * P:(kt + 1) * P].T)  # [P, BH] fp8
            comb[:, OFF[s]:OFF[s] + SU_BYTES] = su.view(np.uint8)
            rank = 0
            for j in range(4):
                if S_CAP[j] <= s:
                    continue
                cj = ctile[j]
                a0 = OFF[s] + SU_BYTES + AT_BYTES * rank
                if kt <= cj:
                    tile_bytes = np.ascontiguousarray(
                        A[kt * P:(kt + 1) * P, cj * P:(cj + 1) * P]
                    ).view(np.uint8)
                    comb[:, a0:a0 + AT_BYTES] = tile_bytes
                rank += 1
        in_maps.append({"comb": comb})
        placement.append((r, c))
    return in_maps, placement


def kernel(state, J):
    from concourse.bass_utils import run_bass_kernel_spmd

    if "nc" not in _cache:
        _cache["nc"] = _build_program()
    nc = _cache["nc"]

    in_maps, placement = _make_in_maps(state, J)
    res = run_bass_kernel_spmd(nc, in_maps, list(range(R * C)))

    out = np.zeros(B, dtype=np.float32)
    for core, (r, c) in enumerate(placement):
        out[c * BH:(c + 1) * BH] += res.results[core]["part"].reshape(BH)
    return out


# revision 26
# speedup vs baseline: 1.1691x; 1.1691x over previous
"""Trainium2 Bass kernel: batched Ising energies E_b = s_b^T J s_b.

state: [1024, 2048] float32 in {0,1};  J: [2048, 2048] float32.
Returns energies [1024] float32.

Because s_i^2 = 1, E = s^T J s = s^T A s where A folds the symmetric
part of J into the upper block-triangle (A_ij = J_ij + J_ji for i<j,
A_ii = J_ii, zero below).  That halves both the matmul work and the
J bytes moved; A ships as bf16 (rel err ~2.5e-3, tolerance 2e-2).

Sharding (8 cores): 2 batch halves x 4 column groups.  Core (r, c)
owns column tiles {4j+r : j=0..3} of A for batch half c.  For a single
SPMD program across cores with different triangle supports, slot j
accumulates its ctile over a fixed cap of 4(j+1) contraction tiles
(ragged edge zero-padded on the host) and ktile storage order is
permuted per core so ctile 4j+r's spin rows always sit at position
4j+3 (uniform epilogue addressing).

Per core: A tiles are the stationary operand; spins ship directly as
fp8e4 {-1,+1} (exact, 1 byte) and stream as the moving operand -- the
PE accepts bf16 stationary x fp8 moving, so no on-chip expand at all.
psum[j] = g = A_block^T s.  Epilogue per slot: m = psum[j] * spin rows
(DVE, fp8 operand), then a ones-vector matmul reduces m across
partitions into a [1, 512] psum accumulator; one output DMA.

Both input streams are HOST-INTERLEAVED into ONE DRAM tensor in exact
consumption order (per contraction step: 512 B of spins then the
step's A tiles), so each chunk is a single DMA -- halving the number
of completion-semaphore increments, whose ~120 ns global drain rate
is what actually paces chunk consumability.  Chunks alternate between
the two HWDGE rings; on-chip consumers read dtype-bitcast slices of
the one combined SBUF buffer.  Dummy matmuls warm the PE clock gate
while the first chunk lands.
"""

import sys

if "/opt/trn_rl_repo" not in sys.path:
    sys.path.insert(0, "/opt/trn_rl_repo")

import numpy as np
import ml_dtypes

B, N = 1024, 2048
P = 128
KT = N // P          # 16 contraction/column tiles
R, C = 4, 2          # column groups x batch halves
BH = B // C          # 512 samples per core
S_CAP = [4, 8, 12, 16]   # per-slot ktile caps (uniform across cores)
N_WARM = 70          # PE clock-gate warmup matmuls
WARM_N = 64
CHUNKS = [(0, 2), (2, 6), (6, 16)]   # step ranges per input DMA

SU_BYTES = BH                        # fp8 spins per step per partition
AT_BYTES = 2 * P                     # one bf16 A tile per partition


def _n_act(s):
    return sum(1 for cap in S_CAP if cap > s)


OFF = [0]
for _s in range(KT):
    OFF.append(OFF[-1] + SU_BYTES + AT_BYTES * _n_act(_s))
TOTAL = OFF[KT]

_cache = {}


def _build_program():
    import concourse.bacc as bacc
    import concourse.mybir as mybir
    import concourse.tile as tile

    bf16 = mybir.dt.bfloat16
    f32 = mybir.dt.float32
    fp8 = mybir.dt.float8e4
    u8 = mybir.dt.uint8

    nc = bacc.Bacc("TRN2", target_bir_lowering=False, debug=False, num_devices=R * C)

    comb_ext = nc.dram_tensor("comb", [P, TOTAL], u8, kind="ExternalInput").ap()
    out_ext = nc.dram_tensor("part", [1, BH], f32, kind="ExternalOutput").ap()

    with tile.TileContext(nc) as tc:
        with (
            tc.tile_pool(name="persist", bufs=1) as persist,
            tc.tile_pool(name="work", bufs=1) as work,
            tc.tile_pool(name="psum", bufs=1, space="PSUM") as psum_pool,
            tc.tile_pool(name="warmps", bufs=1, space="PSUM") as warm_pool,
        ):
            comb_t = persist.tile([P, TOTAL], u8)
            ones_t = persist.tile([P, 1], bf16)
            warm_src = persist.tile([P, P], bf16)
            red_sb = persist.tile([1, BH], f32)
            m_t = [work.tile([P, BH], bf16, name=f"m_{j}") for j in range(4)]

            ps = [psum_pool.tile([P, BH], f32, name=f"ps_{j}") for j in range(4)]
            ep = psum_pool.tile([1, BH], f32, name="ep")
            warm_ps = warm_pool.tile([P, WARM_N], f32)

            def su_view(s):
                return comb_t[:, OFF[s]:OFF[s] + SU_BYTES].bitcast(fp8)

            def a_view(s, rank):
                a0 = OFF[s] + SU_BYTES + AT_BYTES * rank
                return comb_t[:, a0:a0 + AT_BYTES].bitcast(bf16)

            # constants via gpsimd (released early in the preamble)
            nc.gpsimd.memset(warm_src[:], 0.0)
            nc.gpsimd.memset(ones_t[:], 1.0)

            # PE warmup: dummy matmuls keep the HAM activity window busy
            # while the first input chunk lands.
            for _ in range(N_WARM):
                nc.tensor.matmul(
                    warm_ps, lhsT=warm_src[:, :P], rhs=warm_src[:, :WARM_N],
                    start=True, stop=True,
                )

            # one DMA per chunk, alternating HWDGE rings, consumption
            # order.  The first (latency-critical) chunk is split by
            # partition halves across BOTH rings: each half uses 8 SDMA
            # engines, so the slowest-engine completion skew shrinks and
            # both rings push its bytes concurrently.
            (k0, k1) = CHUNKS[0]
            nc.sync.dma_start(
                out=comb_t[0:64, OFF[k0]:OFF[k1]],
                in_=comb_ext[0:64, OFF[k0]:OFF[k1]],
                single_packet=True,
            )
            nc.scalar.dma_start(
                out=comb_t[64:128, OFF[k0]:OFF[k1]],
                in_=comb_ext[64:128, OFF[k0]:OFF[k1]],
                single_packet=True,
            )
            for ci, (k0, k1) in enumerate(CHUNKS[1:]):
                eng = nc.scalar if ci % 2 == 0 else nc.sync
                eng.dma_start(
                    out=comb_t[:, OFF[k0]:OFF[k1]],
                    in_=comb_ext[:, OFF[k0]:OFF[k1]],
                    single_packet=True,
                )

            def epilogue(j):
                # m = ps[j] * spin rows of ctile j (position 4j+3), then
                # reduce across partitions via a ones-vector matmul.
                nc.vector.scalar_tensor_tensor(
                    m_t[j][:],
                    ps[j][:],
                    1.0,
                    su_view(4 * j + 3),
                    mybir.AluOpType.mult,
                    mybir.AluOpType.mult,
                )
                nc.tensor.matmul(
                    ep,
                    lhsT=ones_t[:],
                    rhs=m_t[j][:],
                    start=(j == 0),
                    stop=(j == 3),
                )
                if j == 3:
                    # PSUM -> SBUF (DMA cannot read PSUM)
                    nc.vector.tensor_scalar(
                        red_sb[:], ep[:], 1.0, 0.0,
                        mybir.AluOpType.mult, mybir.AluOpType.add,
                    )

            for s in range(KT):
                rank = 0
                for j in range(4):
                    if S_CAP[j] <= s:
                        continue
                    nc.tensor.matmul(
                        ps[j],
                        lhsT=a_view(s, rank),
                        rhs=su_view(s),
                        start=(s == 0),
                        stop=(s == S_CAP[j] - 1),
                    )
                    rank += 1
                # emit each finished slot's epilogue one step late so the
                # ones-matmul's wait on the DVE multiply is already
                # satisfied when the PE reaches it (no exposed gap).
                for j in range(4):
                    if s == min(S_CAP[j], KT - 1):
                        epilogue(j)

            nc.sync.dma_start(out=out_ext, in_=red_sb[:])

    nc.compile()
    return nc


def _make_in_maps(state, J):
    bf16 = ml_dtypes.bfloat16
    fp8 = ml_dtypes.float8_e4m3
    state = np.asarray(state, dtype=np.float32)
    J = np.asarray(J, dtype=np.float32)

    # fold the symmetric part into the upper block-triangle
    A = np.triu(J + J.T, 1) + np.diag(np.diag(J))
    A = A.astype(bf16)
    sp8 = np.where(state > 0.5, np.float32(1.0), np.float32(-1.0)).astype(fp8)

    in_maps = []
    placement = []
    for core in range(R * C):
        r, c = divmod(core, C)
        kt_of_pos = []
        for g in range(4):
            grp = [x for x in range(4 * g, 4 * g + 4) if x != 4 * g + r]
            kt_of_pos += grp + [4 * g + r]
        ctile = [4 * j + r for j in range(4)]

        sm = sp8[c * BH:(c + 1) * BH]               # [BH, N]
        comb = np.zeros((P, TOTAL), dtype=np.uint8)
        for s in range(KT):
            kt = kt_of_pos[s]
            su = np.ascontiguousarray(sm[:, kt * P:(kt + 1) * P].T)  # [P, BH] fp8
            comb[:, OFF[s]:OFF[s] + SU_BYTES] = su.view(np.uint8)
            rank = 0
            for j in range(4):
                if S_CAP[j] <= s:
                    continue
                cj = ctile[j]
                a0 = OFF[s] + SU_BYTES + AT_BYTES * rank
                if kt <= cj:
                    tile_bytes = np.ascontiguousarray(
                        A[kt * P:(kt + 1) * P, cj * P:(cj + 1) * P]
                    ).view(np.uint8)
                    comb[:, a0:a0 + AT_BYTES] = tile_bytes
                rank += 1
        in_maps.append({"comb": comb})
        placement.append((r, c))
    return in_maps, placement


def kernel(state, J):
    from concourse.bass_utils import run_bass_kernel_spmd

    if "nc" not in _cache:
        _cache["nc"] = _build_program()
    nc = _cache["nc"]

    in_maps, placement = _make_in_maps(state, J)
    res = run_bass_kernel_spmd(nc, in_maps, list(range(R * C)))

    out = np.zeros(B, dtype=np.float32)
    for core, (r, c) in enumerate(placement):
        out[c * BH:(c + 1) * BH] += res.results[core]["part"].reshape(BH)
    return out


# revision 27
# speedup vs baseline: 1.2299x; 1.0520x over previous
"""Trainium2 Bass kernel: batched Ising energies E_b = s_b^T J s_b.

state: [1024, 2048] float32 in {0,1};  J: [2048, 2048] float32.
Returns energies [1024] float32.

Because s_i^2 = 1, E = s^T J s = s^T A s where A folds the symmetric
part of J into the upper block-triangle (A_ij = J_ij + J_ji for i<j,
A_ii = J_ii, zero below).  That halves both the matmul work and the
J bytes moved; A ships as bf16 (rel err ~2.5e-3, tolerance 2e-2).

Sharding (8 cores): 2 batch halves x 4 column groups.  Core (r, c)
owns column tiles {4j+r : j=0..3} of A for batch half c.  For a single
SPMD program across cores with different triangle supports, slot j
accumulates its ctile over a fixed cap of 4(j+1) contraction tiles
(ragged edge zero-padded on the host) and ktile storage order is
permuted per core so ctile 4j+r's spin rows always sit at position
4j+3 (uniform epilogue addressing).

Per core: A tiles are the stationary operand; spins ship directly as
fp8e4 {-1,+1} (exact, 1 byte) and stream as the moving operand -- the
PE accepts bf16 stationary x fp8 moving, so no on-chip expand at all.
psum[j] = g = A_block^T s.  Epilogue per slot: m = psum[j] * spin rows
(DVE, fp8 operand), then a ones-vector matmul reduces m across
partitions into a [1, 512] psum accumulator; one output DMA.

Both input streams are HOST-INTERLEAVED into ONE DRAM tensor in exact
consumption order (per contraction step: 512 B of spins then the
step's A tiles), so each chunk is a single DMA -- halving the number
of completion-semaphore increments, whose ~120 ns global drain rate
is what actually paces chunk consumability.  Chunks alternate between
the two HWDGE rings; on-chip consumers read dtype-bitcast slices of
the one combined SBUF buffer.  Dummy matmuls warm the PE clock gate
while the first chunk lands.
"""

import sys

if "/opt/trn_rl_repo" not in sys.path:
    sys.path.insert(0, "/opt/trn_rl_repo")

import numpy as np
import ml_dtypes

B, N = 1024, 2048
P = 128
KT = N // P          # 16 contraction/column tiles
R, C = 4, 2          # column groups x batch halves
BH = B // C          # 512 samples per core
S_CAP = [4, 8, 12, 16]   # per-slot ktile caps (uniform across cores)
N_WARM = 70          # PE clock-gate warmup matmuls
WARM_N = 64
CHUNKS = [(0, 2), (2, 6), (6, 16)]   # step ranges per input DMA

SU_BYTES = BH                        # fp8 spins per step per partition
AT_BYTES = 2 * P                     # one bf16 A tile per partition


def _n_act(s):
    return sum(1 for cap in S_CAP if cap > s)


OFF = [0]
for _s in range(KT):
    OFF.append(OFF[-1] + SU_BYTES + AT_BYTES * _n_act(_s))
TOTAL = OFF[KT]

_cache = {}


def _build_program():
    import concourse.bacc as bacc
    import concourse.mybir as mybir
    import concourse.tile as tile

    bf16 = mybir.dt.bfloat16
    f32 = mybir.dt.float32
    fp8 = mybir.dt.float8e4
    u8 = mybir.dt.uint8

    nc = bacc.Bacc("TRN2", target_bir_lowering=False, debug=False, num_devices=R * C)

    comb_ext = nc.dram_tensor("comb", [P, TOTAL], u8, kind="ExternalInput").ap()
    out_ext = nc.dram_tensor("part", [1, BH], f32, kind="ExternalOutput").ap()

    with tile.TileContext(nc) as tc:
        with (
            tc.tile_pool(name="persist", bufs=1) as persist,
            tc.tile_pool(name="work", bufs=1) as work,
            tc.tile_pool(name="psum", bufs=1, space="PSUM") as psum_pool,
            tc.tile_pool(name="warmps", bufs=1, space="PSUM") as warm_pool,
        ):
            comb_t = persist.tile([P, TOTAL], u8)
            ones_t = persist.tile([P, 1], bf16)
            warm_src = persist.tile([P, P], bf16)
            red_sb = persist.tile([1, BH], f32)
            m_t = [work.tile([P, BH], bf16, name=f"m_{j}") for j in range(4)]

            ps = [psum_pool.tile([P, BH], f32, name=f"ps_{j}") for j in range(4)]
            ep = psum_pool.tile([1, BH], f32, name="ep")
            warm_ps = warm_pool.tile([P, WARM_N], f32)

            def su_view(s):
                return comb_t[:, OFF[s]:OFF[s] + SU_BYTES].bitcast(fp8)

            def a_view(s, rank):
                a0 = OFF[s] + SU_BYTES + AT_BYTES * rank
                return comb_t[:, a0:a0 + AT_BYTES].bitcast(bf16)

            # constants via gpsimd (released early in the preamble)
            nc.gpsimd.memset(warm_src[:], 0.0)
            nc.gpsimd.memset(ones_t[:], 1.0)

            # PE warmup: dummy matmuls keep the HAM activity window busy
            # while the first input chunk lands.
            for _ in range(N_WARM):
                nc.tensor.matmul(
                    warm_ps, lhsT=warm_src[:, :P], rhs=warm_src[:, :WARM_N],
                    start=True, stop=True,
                )

            # one DMA per chunk, alternating HWDGE rings, consumption order
            for ci, (k0, k1) in enumerate(CHUNKS):
                eng = nc.sync if ci % 2 == 0 else nc.scalar
                eng.dma_start(
                    out=comb_t[:, OFF[k0]:OFF[k1]],
                    in_=comb_ext[:, OFF[k0]:OFF[k1]],
                    single_packet=True,
                )

            def epilogue(j):
                # m = ps[j] * spin rows of ctile j (position 4j+3), then
                # reduce across partitions via a ones-vector matmul.
                nc.vector.scalar_tensor_tensor(
                    m_t[j][:],
                    ps[j][:],
                    1.0,
                    su_view(4 * j + 3),
                    mybir.AluOpType.mult,
                    mybir.AluOpType.mult,
                )
                nc.tensor.matmul(
                    ep,
                    lhsT=ones_t[:],
                    rhs=m_t[j][:],
                    start=(j == 0),
                    stop=(j == 3),
                )
                if j == 3:
                    # PSUM -> SBUF (DMA cannot read PSUM)
                    nc.vector.tensor_scalar(
                        red_sb[:], ep[:], 1.0, 0.0,
                        mybir.AluOpType.mult, mybir.AluOpType.add,
                    )

            for s in range(KT):
                rank = 0
                for j in range(4):
                    if S_CAP[j] <= s:
                        continue
                    nc.tensor.matmul(
                        ps[j],
                        lhsT=a_view(s, rank),
                        rhs=su_view(s),
                        start=(s == 0),
                        stop=(s == S_CAP[j] - 1),
                    )
                    rank += 1
                # emit each finished slot's epilogue one step late so the
                # ones-matmul's wait on the DVE multiply is already
                # satisfied when the PE reaches it (no exposed gap).
                for j in range(4):
                    if s == min(S_CAP[j], KT - 1):
                        epilogue(j)

            nc.sync.dma_start(out=out_ext, in_=red_sb[:])

    nc.compile()
    return nc


def _make_in_maps(state, J):
    bf16 = ml_dtypes.bfloat16
    fp8 = ml_dtypes.float8_e4m3
    state = np.asarray(state, dtype=np.float32)
    J = np.asarray(J, dtype=np.float32)

    # fold the symmetric part into the upper block-triangle
    A = np.triu(J + J.T, 1) + np.diag(np.diag(J))
    A = A.astype(bf16)
    sp8 = np.where(state > 0.5, np.float32(1.0), np.float32(-1.0)).astype(fp8)

    in_maps = []
    placement = []
    for core in range(R * C):
        r, c = divmod(core, C)
        kt_of_pos = []
        for g in range(4):
            grp = [x for x in range(4 * g, 4 * g + 4) if x != 4 * g + r]
            kt_of_pos += grp + [4 * g + r]
        ctile = [4 * j + r for j in range(4)]

        sm = sp8[c * BH:(c + 1) * BH]               # [BH, N]
        comb = np.zeros((P, TOTAL), dtype=np.uint8)
        for s in range(KT):
            kt = kt_of_pos[s]
            su = np.ascontiguousarray(sm[:, kt * P:(kt + 1) * P].T)  # [P, BH] fp8
            comb[:, OFF[s]:OFF[s] + SU_BYTES] = su.view(np.uint8)
            rank = 0
            for j in range(4):
                if S_CAP[j] <= s:
                    continue
                cj = ctile[j]
                a0 = OFF[s] + SU_BYTES + AT_BYTES * rank
                if kt <= cj:
                    tile_bytes = np.ascontiguousarray(
                        A[kt * P:(kt + 1) * P, cj * P:(cj + 1) * P]
                    ).view(np.uint8)
                    comb[:, a0:a0 + AT_BYTES] = tile_bytes
                rank += 1
        in_maps.append({"comb": comb})
        placement.append((r, c))
    return in_maps, placement


def kernel(state, J):
    from concourse.bass_utils import run_bass_kernel_spmd

    if "nc" not in _cache:
        _cache["nc"] = _build_program()
    nc = _cache["nc"]

    in_maps, placement = _make_in_maps(state, J)
    res = run_bass_kernel_spmd(nc, in_maps, list(range(R * C)))

    out = np.zeros(B, dtype=np.float32)
    for core, (r, c) in enumerate(placement):
        out[c * BH:(c + 1) * BH] += res.results[core]["part"].reshape(BH)
    return out
